# revision 5
# baseline (speedup 1.0000x reference)
"""CrossViewTransformer Bass kernel for 8 trn2 NeuronCores.

Problem (per batch element b of 4):
    q = (Wq @ top_b + bq)      # [32, 4096]
    k = (Wk @ side_b + bk)     # [32, 4096]
    v = (Wv @ side_b + bv)     # [256, 4096]
    E = softmax_over_keys(q.T @ k)        # [4096q, 4096k]
    out_b = top_b + (E @ v.T).T           # [256, 4096]

Sharding: 8 cores = (batch b = core//2) x (query half h = core%2).
Each core handles 2048 queries against all 4096 keys of its batch
element; no collectives. Weights replicated.

Per-core kernel (Tile framework):
  - projections: k, vT in f32r (TF32-like, 1 cyc/row on PE); q in fp32.
    vT is computed transposed ([keys, C]) directly by using side as the
    stationary operand, and augmented with a ones column so the E@v
    matmul also produces the softmax row-sums for free. bv is folded in
    via a rank-1 (K=1) accumulating matmul; bq/bk via per-partition
    tensor_scalar adds.
  - attention: softmax without max-subtraction (|scores| <~ 35 for these
    inputs; exp and the fp32 row-sum are safe). For each 512-query chunk
    and each pair of 128-key blocks: scoresT = k_blk.T @ q_chunk on PE
    (f32r), exp on ScalarE (PSUM -> SBUF f32r), then E.T-as-weights
    matmuls accumulate [128q, 256C | rowsum] in PSUM over all 32 key
    blocks.
  - epilogue: recip(rowsum), per-partition scale, PE transpose back to
    [C, q], fp32 residual add with topview, DMA out.
"""

import sys

import numpy as np

B, C, H, W = 4, 256, 64, 64
N = H * W      # 4096 keys per batch element
C8 = 32
NCORES = 8
NQ = N // 2    # 2048 queries per core
QC = 512       # query chunk
QB = 128       # query block (matmul M)
KB = 128       # key block
NKB = N // KB  # 32 key blocks
NCHUNK = NQ // QC  # 4

_BUILT = None


def _build():
    for p in ("/opt/trn_rl_repo", "/root/.axon_site/_ro/trn_rl_repo"):
        if p not in sys.path:
            sys.path.append(p)
    import concourse.bass as bass
    import concourse.tile as tile
    from concourse import bacc, mybir
    from concourse.masks import make_identity

    fp32 = mybir.dt.float32
    f32r = mybir.dt.float32r
    EXP = mybir.ActivationFunctionType.Exp

    nc = bacc.Bacc("TRN2", target_bir_lowering=False, debug=False,
                   num_devices=NCORES)

    top_d = nc.dram_tensor("top", [C, NQ], fp32, kind="ExternalInput").ap()
    side_d = nc.dram_tensor("side", [C, N], fp32, kind="ExternalInput").ap()
    wqT_d = nc.dram_tensor("wqT", [C, C8], fp32, kind="ExternalInput").ap()
    wkT_d = nc.dram_tensor("wkT", [C, C8], fp32, kind="ExternalInput").ap()
    wvT_d = nc.dram_tensor("wvT", [C, C], fp32, kind="ExternalInput").ap()
    bq_d = nc.dram_tensor("bq", [C8, 1], fp32, kind="ExternalInput").ap()
    bk_d = nc.dram_tensor("bk", [C8, 1], fp32, kind="ExternalInput").ap()
    bv_d = nc.dram_tensor("bv", [1, C], fp32, kind="ExternalInput").ap()
    out_d = nc.dram_tensor("out", [C, NQ], fp32, kind="ExternalOutput").ap()

    # channel dim split into 2 partition blocks of 128
    top_r3 = top_d.rearrange("(t p) n -> p t n", p=128)
    side_r3 = side_d.rearrange("(t p) n -> p t n", p=128)
    wqT_r3 = wqT_d.rearrange("(t p) m -> p t m", p=128)
    wkT_r3 = wkT_d.rearrange("(t p) m -> p t m", p=128)
    wvT_r3 = wvT_d.rearrange("(t p) m -> p t m", p=128)
    out_r3 = out_d.rearrange("(t p) n -> p t n", p=128)

    with tile.TileContext(nc) as tc:
        with tc.tile_pool(name="persist", bufs=1) as pers, \
             tc.tile_pool(name="work", bufs=1) as work:

            # ---- persistent SBUF tiles ----
            top_sb = pers.tile([128, 2, NQ], fp32, tag="top")
            side_q = pers.tile([128, 2, N], f32r, tag="side_r")  # rounded
            q_r = pers.tile([C8, NQ], f32r, tag="q")
            k_r = pers.tile([C8, N], f32r, tag="k")
            vT_r = pers.tile([128, NKB, C + 2], f32r, tag="vT")
            out_sb = pers.tile([128, 2, NQ], fp32, tag="out")
            wq_sb = pers.tile([128, 2, C8], fp32, tag="wq")
            wk_r = pers.tile([128, 2, C8], f32r, tag="wk")
            wv_r = pers.tile([128, 2, C], f32r, tag="wv")
            bq_sb = pers.tile([C8, 1], fp32, tag="bq")
            bk_sb = pers.tile([C8, 1], fp32, tag="bk")
            bv_r = pers.tile([1, C], f32r, tag="bv")
            ones_r = pers.tile([1, 128], f32r, tag="ones")
            ident = pers.tile([128, 128], fp32, tag="ident")

            nc.gpsimd.memset(ones_r[:].bitcast(fp32), 1.0)
            nc.gpsimd.memset(vT_r[:, :, C:C + 2].bitcast(fp32), 0.0)
            nc.gpsimd.memset(vT_r[:, :, C:C + 1].bitcast(fp32), 1.0)
            make_identity(nc, ident[:])

            # ---- loads (weights + top directly; side via rounding) ----
            nc.sync.dma_start(top_sb[:], top_r3[:])
            nc.sync.dma_start(wq_sb[:], wqT_r3[:])

            with tc.tile_pool(name="stage", bufs=1) as stage:
                wk_f = stage.tile([128, 2, C8], fp32, tag="wk_f")
                wv_f = stage.tile([128, 2, C], fp32, tag="wv_f")
                bv_f = stage.tile([1, C], fp32, tag="bv_f")
                nc.sync.dma_start(wk_f[:], wkT_r3[:])
                nc.sync.dma_start(wv_f[:], wvT_r3[:])
                nc.sync.dma_start(bq_sb[:], bq_d[:])
                nc.sync.dma_start(bk_sb[:], bk_d[:])
                nc.sync.dma_start(bv_f[:], bv_d[:])
                nc.vector.tensor_copy(wk_r[:], wk_f[:])
                nc.vector.tensor_copy(wv_r[:], wv_f[:])
                nc.vector.tensor_copy(bv_r[:], bv_f[:])

                side_f = stage.tile([128, 2, N], fp32, tag="side_f")
                NLOAD = 4
                for s in range(NLOAD):
                    sl = bass.ts(s, N // NLOAD)
                    nc.sync.dma_start(side_f[:, :, sl], side_r3[:, :, sl])
                    nc.vector.tensor_copy(side_q[:, :, sl], side_f[:, :, sl])

            # ---- projections ----
            with tc.tile_pool(name="ps_proj", bufs=1, space="PSUM") as psp:
                # vT[keys, C] per key block, bv folded via rank-1 matmul
                for j in range(NKB):
                    pv = psp.tile([128, C], fp32, tag="pj", bufs=2)
                    jsl = bass.ts(j, KB)
                    nc.tensor.matmul(pv[:], side_q[:, 0, jsl], wv_r[:, 0, :],
                                     start=True, stop=False)
                    nc.tensor.matmul(pv[:], side_q[:, 1, jsl], wv_r[:, 1, :],
                                     start=False, stop=False)
                    nc.tensor.matmul(pv[:], ones_r[:], bv_r[:],
                                     start=False, stop=True)
                    nc.vector.tensor_copy(vT_r[:, j, 0:C], pv[:])

                # k = Wk @ side + bk   (f32r), 8 slices of 512
                for s in range(N // 512):
                    pk = psp.tile([C8, 512], fp32, tag="pj", bufs=2)
                    sl = bass.ts(s, 512)
                    nc.tensor.matmul(pk[:], wk_r[:, 0, :], side_q[:, 0, sl],
                                     start=True, stop=False)
                    nc.tensor.matmul(pk[:], wk_r[:, 1, :], side_q[:, 1, sl],
                                     start=False, stop=True)
                    nc.vector.tensor_scalar_add(k_r[:, sl], pk[:], bk_sb[:])

                # q = Wq @ top + bq   (fp32 matmul), 4 slices of 512
                for s in range(NQ // 512):
                    pq = psp.tile([C8, 512], fp32, tag="pj", bufs=2)
                    sl = bass.ts(s, 512)
                    nc.tensor.matmul(pq[:], wq_sb[:, 0, :], top_sb[:, 0, sl],
                                     start=True, stop=False)
                    nc.tensor.matmul(pq[:], wq_sb[:, 1, :], top_sb[:, 1, sl],
                                     start=False, stop=True)
                    nc.vector.tensor_scalar_add(q_r[:, sl], pq[:], bq_sb[:])

            # ---- attention ----
            with tc.tile_pool(name="ps_attn", bufs=1, space="PSUM") as psa:
                for qc in range(NCHUNK):
                    qsl = bass.ts(qc, QC)
                    av = [psa.tile([128, C + 2], fp32, tag="av", bufs=4,
                                   name=f"av{qc}_{i}")
                          for i in range(QC // QB)]
                    for jp in range(NKB // 2):
                        sc = psa.tile([128, 2, 512], fp32, tag="sc", bufs=2)
                        ex = work.tile([128, 2, 512], f32r, tag="ex", bufs=3)
                        for u in range(2):
                            j = 2 * jp + u
                            nc.tensor.matmul(sc[:, u, :],
                                             k_r[:, bass.ts(j, KB)],
                                             q_r[:, qsl],
                                             start=True, stop=True)
                        nc.scalar.activation(ex[:], sc[:], EXP)
                        for u in range(2):
                            j = 2 * jp + u
                            for qb in range(QC // QB):
                                nc.tensor.matmul(
                                    av[qb][:],
                                    ex[:, u, bass.ts(qb, QB)],
                                    vT_r[:, j, :],
                                    start=(j == 0), stop=(j == NKB - 1))
                    # epilogue: normalize, transpose, residual
                    for qb in range(QC // QB):
                        q0 = qc * QC + qb * QB
                        rc = work.tile([128, 1], fp32, tag="rc", bufs=2)
                        nc.vector.reciprocal(rc[:], av[qb][:, C:C + 1])
                        sca = work.tile([128, C], fp32, tag="sca", bufs=2)
                        nc.vector.tensor_scalar_mul(sca[:], av[qb][:, 0:C],
                                                    rc[:])
                        for t in range(2):
                            tp = psa.tile([128, 128], fp32, tag="av",
                                          bufs=4, name=f"tp{qc}_{qb}_{t}")
                            nc.tensor.transpose(tp[:], sca[:, bass.ts(t, 128)],
                                                ident[:])
                            nc.vector.tensor_add(
                                out_sb[:, t, q0:q0 + QB], tp[:],
                                top_sb[:, t, q0:q0 + QB])
                    for t in range(2):
                        nc.sync.dma_start(out_r3[:, t, qsl],
                                          out_sb[:, t, qsl])

    nc.compile()
    return nc


def _get_built():
    global _BUILT
    if _BUILT is None:
        _BUILT = _build()
    return _BUILT


def kernel(topview, sideview, Wq, bq, Wk, bk, Wv, bv):
    from concourse.bass_utils import run_bass_kernel_spmd

    topview = np.asarray(topview, dtype=np.float32)
    sideview = np.asarray(sideview, dtype=np.float32)
    wqT = np.ascontiguousarray(np.asarray(Wq, np.float32).T)
    wkT = np.ascontiguousarray(np.asarray(Wk, np.float32).T)
    wvT = np.ascontiguousarray(np.asarray(Wv, np.float32).T)
    bq = np.asarray(bq, np.float32).reshape(C8, 1)
    bk = np.asarray(bk, np.float32).reshape(C8, 1)
    bv = np.asarray(bv, np.float32).reshape(1, C)

    top_f = topview.reshape(B, C, N)
    side_f = sideview.reshape(B, C, N)

    in_maps = []
    for core in range(NCORES):
        b, h = core // 2, core % 2
        in_maps.append({
            "top": np.ascontiguousarray(top_f[b, :, h * NQ:(h + 1) * NQ]),
            "side": np.ascontiguousarray(side_f[b]),
            "wqT": wqT, "wkT": wkT, "wvT": wvT,
            "bq": bq, "bk": bk, "bv": bv,
        })

    global _last_in_maps
    _last_in_maps = in_maps

    nc = _get_built()
    res = run_bass_kernel_spmd(nc, in_maps, core_ids=list(range(NCORES)))

    out = np.empty((B, C, N), dtype=np.float32)
    for core in range(NCORES):
        b, h = core // 2, core % 2
        out[b, :, h * NQ:(h + 1) * NQ] = res.results[core]["out"]
    return out.reshape(B, C, H, W)


# revision 6
# speedup vs baseline: 1.2111x; 1.2111x over previous
"""CrossViewTransformer Bass kernel for 8 trn2 NeuronCores.

Problem (per batch element b of 4):
    q = (Wq @ top_b + bq)      # [32, 4096]
    k = (Wk @ side_b + bk)     # [32, 4096]
    v = (Wv @ side_b + bv)     # [256, 4096]
    E = softmax_over_keys(q.T @ k)        # [4096q, 4096k]
    out_b = top_b + (E @ v.T).T           # [256, 4096]

Sharding: 8 cores = (batch b = core//2) x (query half h = core%2).
Each core handles 2048 queries against all 4096 keys of its batch
element; no collectives. Weights replicated.

Precision strategy: scores (q/k projections + q.T@k) run in f32r
(TF32-like) so the exp argument is accurate to ~3e-3 abs; the value
path (vT projection, E@v) runs in bf16 — E is a positive softmax
weight so no cancellation amplification — and the residual add with
topview is exact fp32. Softmax skips max-subtraction (|scores| < ~40
for these inputs, safely inside fp32 exp range), which makes the
row-sum a plain linear functional: it is produced by an extra ones
column appended to vT inside the same accumulating matmul.

Per-core pipeline (Tile framework):
  - projections: vT[keys, C] computed transposed directly (side as the
    stationary operand), bv folded in via a rank-1 (K=1) accumulating
    matmul; bq/bk via per-partition tensor_scalar adds on the
    PSUM->SBUF copies.
  - attention, per 512-query chunk, per pair of 128-key blocks:
    scoresT = k_blk.T @ q_chunk on PE (f32r), exp on ScalarE
    (PSUM -> SBUF bf16), then E-as-weights bf16 matmuls accumulate
    [128q, 256C | rowsum] in PSUM over all 32 key blocks.
  - epilogue: recip(rowsum), per-partition scale (bf16), PE transpose
    back to [C, q] (bf16), fp32 residual add with topview, DMA out.
"""

import sys

import numpy as np

B, C, H, W = 4, 256, 64, 64
N = H * W      # 4096 keys per batch element
C8 = 32
NCORES = 8
NQ = N // 2    # 2048 queries per core
QC = 512       # query chunk
QB = 128       # query block (matmul M)
KB = 128       # key block
NKB = N // KB  # 32 key blocks
NCHUNK = NQ // QC  # 4

_BUILT = None


def _build():
    for p in ("/opt/trn_rl_repo", "/root/.axon_site/_ro/trn_rl_repo"):
        if p not in sys.path:
            sys.path.append(p)
    import concourse.bass as bass
    import concourse.tile as tile
    from concourse import bacc, mybir
    from concourse.masks import make_identity

    fp32 = mybir.dt.float32
    f32r = mybir.dt.float32r
    bf16 = mybir.dt.bfloat16
    EXP = mybir.ActivationFunctionType.Exp

    nc = bacc.Bacc("TRN2", target_bir_lowering=False, debug=False,
                   num_devices=NCORES)

    top_d = nc.dram_tensor("top", [C, NQ], fp32, kind="ExternalInput").ap()
    side_d = nc.dram_tensor("side", [C, N], fp32, kind="ExternalInput").ap()
    wqT_d = nc.dram_tensor("wqT", [C, C8], fp32, kind="ExternalInput").ap()
    wkT_d = nc.dram_tensor("wkT", [C, C8], fp32, kind="ExternalInput").ap()
    wvT_d = nc.dram_tensor("wvT", [C, C], fp32, kind="ExternalInput").ap()
    bq_d = nc.dram_tensor("bq", [C8, 1], fp32, kind="ExternalInput").ap()
    bk_d = nc.dram_tensor("bk", [C8, 1], fp32, kind="ExternalInput").ap()
    bv_d = nc.dram_tensor("bv", [1, C], fp32, kind="ExternalInput").ap()
    out_d = nc.dram_tensor("out", [C, NQ], fp32, kind="ExternalOutput").ap()

    # channel dim split into 2 partition blocks of 128
    top_r3 = top_d.rearrange("(t p) n -> p t n", p=128)
    side_r3 = side_d.rearrange("(t p) n -> p t n", p=128)
    wqT_r3 = wqT_d.rearrange("(t p) m -> p t m", p=128)
    wkT_r3 = wkT_d.rearrange("(t p) m -> p t m", p=128)
    wvT_r3 = wvT_d.rearrange("(t p) m -> p t m", p=128)
    out_r3 = out_d.rearrange("(t p) n -> p t n", p=128)

    with tile.TileContext(nc) as tc:
        with tc.tile_pool(name="persist", bufs=1) as pers, \
             tc.tile_pool(name="work", bufs=1) as work:

            # ---- persistent SBUF tiles ----
            top_sb = pers.tile([128, 2, NQ], fp32, tag="top")
            top_r = pers.tile([128, 2, NQ], f32r, tag="top_r")
            side_q = pers.tile([128, 2, N], f32r, tag="side_r")
            side_bf = pers.tile([128, 2, N], bf16, tag="side_bf")
            q_r = pers.tile([C8, NQ], f32r, tag="q")
            k_r = pers.tile([C8, N], f32r, tag="k")
            vT_b = pers.tile([128, NKB, C + 2], bf16, tag="vT")
            out_sb = pers.tile([128, 2, NQ], fp32, tag="out")
            wq_r = pers.tile([128, 2, C8], f32r, tag="wq")
            wk_r = pers.tile([128, 2, C8], f32r, tag="wk")
            wv_b = pers.tile([128, 2, C], bf16, tag="wv")
            bq_sb = pers.tile([C8, 1], fp32, tag="bq")
            bk_sb = pers.tile([C8, 1], fp32, tag="bk")
            bv_b = pers.tile([1, C], bf16, tag="bv")
            ones_b = pers.tile([1, 128], bf16, tag="ones")
            ident = pers.tile([128, 128], bf16, tag="ident")

            nc.gpsimd.memset(ones_b[:], 1.0)
            nc.gpsimd.memset(vT_b[:, :, C:C + 2], 0.0)
            nc.gpsimd.memset(vT_b[:, :, C:C + 1], 1.0)
            make_identity(nc, ident[:])

            # ---- loads; cast staging ----
            nc.sync.dma_start(top_sb[:], top_r3[:])
            nc.vector.tensor_copy(top_r[:], top_sb[:])

            with tc.tile_pool(name="stage", bufs=1) as stage:
                wq_f = stage.tile([128, 2, C8], fp32, tag="wq_f")
                wk_f = stage.tile([128, 2, C8], fp32, tag="wk_f")
                wv_f = stage.tile([128, 2, C], fp32, tag="wv_f")
                bv_f = stage.tile([1, C], fp32, tag="bv_f")
                nc.sync.dma_start(wq_f[:], wqT_r3[:])
                nc.sync.dma_start(wk_f[:], wkT_r3[:])
                nc.sync.dma_start(wv_f[:], wvT_r3[:])
                nc.sync.dma_start(bq_sb[:], bq_d[:])
                nc.sync.dma_start(bk_sb[:], bk_d[:])
                nc.sync.dma_start(bv_f[:], bv_d[:])
                nc.vector.tensor_copy(wq_r[:], wq_f[:])
                nc.vector.tensor_copy(wk_r[:], wk_f[:])
                nc.vector.tensor_copy(wv_b[:], wv_f[:])
                nc.vector.tensor_copy(bv_b[:], bv_f[:])

                side_f = stage.tile([128, 2, N], fp32, tag="side_f")
                NLOAD = 4
                for s in range(NLOAD):
                    sl = bass.ts(s, N // NLOAD)
                    nc.sync.dma_start(side_f[:, :, sl], side_r3[:, :, sl])
                    nc.vector.tensor_copy(side_q[:, :, sl], side_f[:, :, sl])
                    nc.vector.tensor_copy(side_bf[:, :, sl], side_f[:, :, sl])

            # ---- projections ----
            with tc.tile_pool(name="ps_proj", bufs=1, space="PSUM") as psp:
                # vT[keys, C] per key block (bf16), bv via rank-1 matmul
                for j in range(NKB):
                    pv = psp.tile([128, C], fp32, tag="pj", bufs=2,
                                  name=f"pv{j}")
                    jsl = bass.ts(j, KB)
                    nc.tensor.matmul(pv[:], side_bf[:, 0, jsl], wv_b[:, 0, :],
                                     start=True, stop=False)
                    nc.tensor.matmul(pv[:], side_bf[:, 1, jsl], wv_b[:, 1, :],
                                     start=False, stop=False)
                    nc.tensor.matmul(pv[:], ones_b[:], bv_b[:],
                                     start=False, stop=True)
                    nc.vector.tensor_copy(vT_b[:, j, 0:C], pv[:])

                # k = Wk @ side + bk   (f32r), 8 slices of 512
                for s in range(N // 512):
                    pk = psp.tile([C8, 512], fp32, tag="pj", bufs=2,
                                  name=f"pk{s}")
                    sl = bass.ts(s, 512)
                    nc.tensor.matmul(pk[:], wk_r[:, 0, :], side_q[:, 0, sl],
                                     start=True, stop=False)
                    nc.tensor.matmul(pk[:], wk_r[:, 1, :], side_q[:, 1, sl],
                                     start=False, stop=True)
                    nc.vector.tensor_scalar_add(k_r[:, sl], pk[:], bk_sb[:])

                # q = Wq @ top + bq   (f32r), 4 slices of 512
                for s in range(NQ // 512):
                    pq = psp.tile([C8, 512], fp32, tag="pj", bufs=2,
                                  name=f"pq{s}")
                    sl = bass.ts(s, 512)
                    nc.tensor.matmul(pq[:], wq_r[:, 0, :], top_r[:, 0, sl],
                                     start=True, stop=False)
                    nc.tensor.matmul(pq[:], wq_r[:, 1, :], top_r[:, 1, sl],
                                     start=False, stop=True)
                    nc.vector.tensor_scalar_add(q_r[:, sl], pq[:], bq_sb[:])

            # ---- attention ----
            with tc.tile_pool(name="ps_attn", bufs=1, space="PSUM") as psa:
                for qc in range(NCHUNK):
                    qsl = bass.ts(qc, QC)
                    av = [psa.tile([128, C + 2], fp32, tag="av", bufs=4,
                                   name=f"av{qc}_{i}")
                          for i in range(QC // QB)]
                    for jp in range(NKB // 2):
                        sc = psa.tile([128, 2, 512], fp32, tag="sc", bufs=2,
                                      name=f"sc{qc}_{jp}")
                        ex = work.tile([128, 2, 512], bf16, tag="ex", bufs=3,
                                       name=f"ex{qc}_{jp}")
                        for u in range(2):
                            j = 2 * jp + u
                            nc.tensor.matmul(sc[:, u, :],
                                             k_r[:, bass.ts(j, KB)],
                                             q_r[:, qsl],
                                             start=True, stop=True)
                        nc.scalar.activation(ex[:], sc[:], EXP)
                        for u in range(2):
                            j = 2 * jp + u
                            for qb in range(QC // QB):
                                nc.tensor.matmul(
                                    av[qb][:],
                                    ex[:, u, bass.ts(qb, QB)],
                                    vT_b[:, j, :],
                                    start=(j == 0), stop=(j == NKB - 1))
                    # epilogue: normalize, transpose, residual
                    for qb in range(QC // QB):
                        q0 = qc * QC + qb * QB
                        rc = work.tile([128, 1], fp32, tag="rc", bufs=2,
                                       name=f"rc{qc}_{qb}")
                        nc.vector.reciprocal(rc[:], av[qb][:, C:C + 1])
                        sca = work.tile([128, C], bf16, tag="sca", bufs=2,
                                        name=f"sca{qc}_{qb}")
                        nc.vector.tensor_scalar_mul(sca[:], av[qb][:, 0:C],
                                                    rc[:])
                        for t in range(2):
                            tp = psa.tile([128, 128], bf16, tag="av",
                                          bufs=4, name=f"tp{qc}_{qb}_{t}")
                            nc.tensor.transpose(tp[:], sca[:, bass.ts(t, 128)],
                                                ident[:])
                            nc.vector.tensor_add(
                                out_sb[:, t, q0:q0 + QB], tp[:],
                                top_sb[:, t, q0:q0 + QB])
                    for t in range(2):
                        nc.sync.dma_start(out_r3[:, t, qsl],
                                          out_sb[:, t, qsl])

    nc.compile()
    return nc


def _get_built():
    global _BUILT
    if _BUILT is None:
        _BUILT = _build()
    return _BUILT


def kernel(topview, sideview, Wq, bq, Wk, bk, Wv, bv):
    from concourse.bass_utils import run_bass_kernel_spmd

    topview = np.asarray(topview, dtype=np.float32)
    sideview = np.asarray(sideview, dtype=np.float32)
    wqT = np.ascontiguousarray(np.asarray(Wq, np.float32).T)
    wkT = np.ascontiguousarray(np.asarray(Wk, np.float32).T)
    wvT = np.ascontiguousarray(np.asarray(Wv, np.float32).T)
    bq = np.asarray(bq, np.float32).reshape(C8, 1)
    bk = np.asarray(bk, np.float32).reshape(C8, 1)
    bv = np.asarray(bv, np.float32).reshape(1, C)

    top_f = topview.reshape(B, C, N)
    side_f = sideview.reshape(B, C, N)

    in_maps = []
    for core in range(NCORES):
        b, h = core // 2, core % 2
        in_maps.append({
            "top": np.ascontiguousarray(top_f[b, :, h * NQ:(h + 1) * NQ]),
            "side": np.ascontiguousarray(side_f[b]),
            "wqT": wqT, "wkT": wkT, "wvT": wvT,
            "bq": bq, "bk": bk, "bv": bv,
        })

    global _last_in_maps
    _last_in_maps = in_maps

    nc = _get_built()
    res = run_bass_kernel_spmd(nc, in_maps, core_ids=list(range(NCORES)))

    out = np.empty((B, C, N), dtype=np.float32)
    for core in range(NCORES):
        b, h = core // 2, core % 2
        out[b, :, h * NQ:(h + 1) * NQ] = res.results[core]["out"]
    return out.reshape(B, C, H, W)
